# revision 9
# baseline (speedup 1.0000x reference)
"""MoE routing gate kernel for Trainium2 (8 NeuronCores, data-parallel).

Problem (hardcoded): x [4, 4096, 2048] f32, w_gate [64, 2048] f32,
expert_bias [64] f32 (zeros per spec).
  gate_logits = x @ w_gate.T          # [B, S, 64]
  gate_weights = sigmoid(gate_logits)
  topk_vals, topk_idx = top_k(gate_logits + bias, k=8)
  topk_weights = gather(gate_weights, topk_idx); normalize
Returns (topk_weights [4,4096,8] f32, topk_indices [4,4096,8] int32).

Strategy: shard the 16384 tokens across 8 cores (2048 each); replicate
w_gate. The matmul runs as an exact fp16 hi/lo two-pass scheme instead
of native fp32 (4 cycles/row on the PE): host splits
  x = xh + 2^-11 * xl2,  w = wh + 2^-11 * wl2   (all fp16, exact to
~fp32 precision since fp16 carries 11 mantissa bits per level), and the
device computes, per 128-dim contraction chunk k with ONE stationary
load of S_k = [wh_k | wl2_k] (fp16, full 128-wide PE):
  psum[:, 0:256]   += S_k.T @ xh_k    (tok 0..255 of the group)
  psum[:, 256:512] += S_k.T @ xl2_k
i.e. one 512-row fp16 matmul per k at 1 cycle/row (4x faster than
fp32). Reconstruction (all fp32):
  logits = (psum_hi_rows | xh) + 2^-11*(psum_lo_rows | xh)
         + 2^-11*(psum_hi_rows | xl2)   [+ 2^-22 * (lo|xl2), dropped]
The hi-half combine (same psum partitions) is one DVE
scalar_tensor_tensor; the cross-partition term rides along through the
PE transpose and folds in with a second scalar_tensor_tensor in
token-major layout. Per 128-token tile the DVE max/max_index ops give
top-8 values+indices; ACT sigmoid (+fused row-sum accum), DVE
reciprocal and scalar-mul normalize. Expert bias is zeros per the
problem spec (a numpy fallback guards the general case).
"""

import numpy as np

_B, _S, _D, _E = 4, 4096, 2048, 64
_K = 8
_NCORES = 8
_TOK = _B * _S              # 16384 tokens
_TC = _TOK // _NCORES       # 2048 tokens per core
_GT = 256                   # tokens per group (one psum bank: 2*256 f32)
_NG = _TC // _GT            # 8 token groups per core
_NKC = _D // 128            # 16 contraction chunks
_SPLIT = 2048.0             # 2^11 hi/lo split scale

_prog_cache = {}


def _ensure_path():
    import sys
    for p in ("/opt/trn_rl_repo",):
        if p not in sys.path:
            sys.path.insert(0, p)


def _build_program():
    """Per-core Bass/Tile program (SPMD: same program, different data)."""
    _ensure_path()
    import concourse.bass as bass
    import concourse.tile as tile
    from concourse import bacc, mybir

    nc = bacc.Bacc("TRN2", target_bir_lowering=False, debug=False,
                   num_devices=_NCORES)

    f32 = mybir.dt.float32
    f16 = mybir.dt.float16
    u32 = mybir.dt.uint32
    MULT = mybir.AluOpType.mult
    ADD = mybir.AluOpType.add
    C = 1.0 / _SPLIT

    # DRAM I/O (per core). x layout: [g, dp, k, pass, tau] so each
    # 256-token group streams as one fully-contiguous-per-partition
    # block holding both fp16 passes.
    xg = nc.dram_tensor("xg", [_NG, 128, _NKC, 2, _GT], f16,
                        kind="ExternalInput")
    wt = nc.dram_tensor("wt", [128, _NKC, 128], f16, kind="ExternalInput")
    ident = nc.dram_tensor("ident", [128, 128], f32, kind="ExternalInput")
    out_w = nc.dram_tensor("out_w", [128, _NG, 2, _K], f32,
                           kind="ExternalOutput")
    out_i = nc.dram_tensor("out_i", [128, _NG, 2, _K], u32,
                           kind="ExternalOutput")

    # k-chunk split per group's DMA: fine-grained first loads so the PE
    # starts early; coarser afterwards for DMA efficiency.
    g0_chunks = (1, 1, 2, 4, 8)
    gN_chunks = (8, 8)

    with tile.TileContext(nc) as tc:
        with (
            tc.tile_pool(name="xpool", bufs=4) as xpool,
            tc.tile_pool(name="wpool", bufs=1) as wpool,
            tc.tile_pool(name="psM", bufs=2, space=bass.MemorySpace.PSUM) as psM,
            tc.tile_pool(name="psT", bufs=2, space=bass.MemorySpace.PSUM) as psT,
            tc.tile_pool(name="psW", bufs=1, space=bass.MemorySpace.PSUM) as psW,
            tc.tile_pool(name="cpool", bufs=2) as cpool,
            tc.tile_pool(name="lpool", bufs=2) as lpool,
            tc.tile_pool(name="opool", bufs=2) as opool,
            tc.tile_pool(name="tpool", bufs=4) as tpool,
        ):
            # All loads ride the sync ring in dependency order. The k=0
            # weight slice goes first so the opening matmul gates only
            # on it plus the first 128 KiB x chunk.
            # Interleave the weight/ident loads with group 0's x chunks
            # so neither stream serially delays the other's first use.
            wt_sb = wpool.tile([128, _NKC, 128], f16)
            nc.sync.dma_start(wt_sb[:, 0:1], wt[:, 0:1])
            xt0 = xpool.tile([128, _NKC, 2, _GT], f16, tag="xg")
            nc.sync.dma_start(xt0[:, 0:1], xg[0][:, 0:1])
            nc.sync.dma_start(wt_sb[:, 1:4], wt[:, 1:4])
            nc.sync.dma_start(xt0[:, 1:4], xg[0][:, 1:4])
            nc.sync.dma_start(wt_sb[:, 4:], wt[:, 4:])
            nc.sync.dma_start(xt0[:, 4:8], xg[0][:, 4:8])
            id_sb = wpool.tile([128, 128], f32)
            nc.sync.dma_start(id_sb[:], ident[:])
            nc.sync.dma_start(xt0[:, 8:], xg[0][:, 8:])

            # PE p-state warmup: ~3us of dummy matmuls on zeroed SBUF
            # during the otherwise-dead preamble window, so the tensor
            # engine reaches its top clock before the first real matmul
            # (cold PE runs ~1.5-2.3x slower for the first ~3us busy).
            wu_m = wpool.tile([128, 512], f16)
            nc.gpsimd.memset(wu_m[:], 0.0)
            wu_s = wpool.tile([128, _E], f16)
            nc.gpsimd.memset(wu_s[:], 0.0)
            ps_wu = psW.tile([_E, 512], f32, tag="wu")
            for _ in range(8):
                nc.tensor.matmul(ps_wu[:], wu_s[:], wu_m[:],
                                 start=True, stop=True)

            for g in range(_NG):
                xt = xt0 if g == 0 else xpool.tile([128, _NKC, 2, _GT], f16,
                                                   tag="xg")
                if g > 0:
                    k0 = 0
                    for nk in gN_chunks:
                        nc.sync.dma_start(xt[:, k0:k0 + nk],
                                          xg[g][:, k0:k0 + nk])
                        k0 += nk

                # One 512-row fp16 matmul per k: [xh tok | xl2 tok].
                ps = psM.tile([_E * 2, 2 * _GT], f32, tag="ps")
                for k in range(_NKC):
                    nc.tensor.matmul(
                        ps[:], wt_sb[:, k], xt[:, k],
                        start=(k == 0), stop=(k == _NKC - 1),
                    )

                # cmb rows 0:64  = wh.x  (hi|xh + 2^-11 * hi|xl2)
                # cmb rows 64:128 = wl2.xh (folds in post-transpose)
                # (walrus: at most one PSUM input per DVE op, so stage
                # A0 through SBUF on the scalar engine first)
                cmb = cpool.tile([128, _GT], f32, tag="cmb")
                a0 = cpool.tile([_E, _GT], f32, tag="a0")
                nc.scalar.copy(a0[:], ps[0:_E, 0:_GT])
                nc.vector.scalar_tensor_tensor(
                    cmb[0:_E, :], ps[0:_E, _GT:], C, a0[:],
                    MULT, ADD,
                )
                nc.scalar.copy(cmb[_E:, :], ps[_E:, 0:_GT])

                pt = psT.tile([128, 2, 128], f32, tag="pt")
                asb = lpool.tile([128, 2, _E], f32, tag="asb")
                lg = lpool.tile([128, 2, _E], f32, tag="lg")
                wg = opool.tile([128, 2, _K], f32, tag="wg")
                ig = opool.tile([128, 2, _K], u32, tag="ig")
                for j in range(2):
                    nc.tensor.transpose(
                        pt[:, j], cmb[:, bass.ts(j, 128)], id_sb[:],
                    )
                    nc.scalar.copy(asb[:, j], pt[:, j, 0:_E])
                    nc.vector.scalar_tensor_tensor(
                        lg[:, j], pt[:, j, _E:], C, asb[:, j],
                        MULT, ADD,
                    )
                    vals = tpool.tile([128, _K], f32, tag="vals")
                    nc.vector.max(vals[:], lg[:, j])
                    nc.vector.max_index(ig[:, j], vals[:], lg[:, j])

                    sig = tpool.tile([128, _K], f32, tag="sig")
                    nc.scalar.activation(
                        sig[:], vals[:], mybir.ActivationFunctionType.Sigmoid,
                    )
                    ssum = tpool.tile([128, 1], f32, tag="ssum")
                    nc.vector.reduce_sum(
                        ssum[:], sig[:], axis=mybir.AxisListType.X,
                    )
                    rsum = tpool.tile([128, 1], f32, tag="rsum")
                    nc.vector.reciprocal(rsum[:], ssum[:])
                    nc.vector.tensor_scalar_mul(wg[:, j], sig[:], rsum[:])

                nc.scalar.dma_start(out_w[:, g], wg[:])
                nc.scalar.dma_start(out_i[:, g], ig[:])

    nc.compile()
    return nc


def _get_program():
    if "p" not in _prog_cache:
        _prog_cache["p"] = _build_program()
    return _prog_cache["p"]


def _split_f16(a):
    """a (f32) -> (hi, lo2) fp16 with a ~= hi + lo2/2048 (near-exact)."""
    hi = a.astype(np.float16)
    lo2 = ((a - hi.astype(np.float32)) * _SPLIT).astype(np.float16)
    return hi, lo2


def _pack_inputs(x, w_gate):
    """Host-side layout transform. Returns per-core input maps."""
    x2 = np.ascontiguousarray(x, dtype=np.float32).reshape(_TOK, _D)
    w = np.asarray(w_gate, dtype=np.float32)

    wh, wl2 = _split_f16(w)
    # wt[dp, k, 0:64] = wh[e, k*128+dp]; [.., 64:128] = wl2
    wt = np.empty((128, _NKC, 128), np.float16)
    wt[:, :, 0:_E] = wh.T.reshape(_NKC, 128, _E).transpose(1, 0, 2)
    wt[:, :, _E:] = wl2.T.reshape(_NKC, 128, _E).transpose(1, 0, 2)
    ident = np.eye(128, dtype=np.float32)

    xh, xl2 = _split_f16(x2)

    def pk(a, c):  # core slice -> [g, dp, k, tau]
        ac = a[c * _TC:(c + 1) * _TC]
        return ac.reshape(_NG, _GT, _NKC, 128).transpose(0, 3, 2, 1)

    in_maps = []
    for c in range(_NCORES):
        xgc = np.empty((_NG, 128, _NKC, 2, _GT), np.float16)
        xgc[:, :, :, 0] = pk(xh, c)
        xgc[:, :, :, 1] = pk(xl2, c)
        in_maps.append({"xg": np.ascontiguousarray(xgc), "wt": wt,
                        "ident": ident})
    return in_maps


def _unpack_outputs(results):
    w_parts, i_parts = [], []
    for r in results:
        # [128 tau, 8 g, 2 j, 8] -> token (2g+j)*128+tau -> [2048, 8]
        w_parts.append(
            r["out_w"].reshape(128, _NG * 2, _K).transpose(1, 0, 2).reshape(_TC, _K)
        )
        i_parts.append(
            r["out_i"].reshape(128, _NG * 2, _K).transpose(1, 0, 2).reshape(_TC, _K)
        )
    weights = np.concatenate(w_parts, axis=0).reshape(_B, _S, _K)
    indices = (
        np.concatenate(i_parts, axis=0).astype(np.int32).reshape(_B, _S, _K)
    )
    return weights, indices


def _numpy_reference(x, w_gate, expert_bias):
    """Exact fallback for the (unspecced) nonzero-bias case."""
    x2 = np.asarray(x, dtype=np.float32).reshape(_TOK, _D)
    logits = x2 @ np.asarray(w_gate, dtype=np.float32).T
    gw = 1.0 / (1.0 + np.exp(-logits))
    biased = logits + np.asarray(expert_bias, dtype=np.float32)
    idx = np.argsort(-biased, axis=-1, kind="stable")[:, :_K].astype(np.int32)
    tw = np.take_along_axis(gw, idx, axis=-1)
    tw = tw / tw.sum(axis=-1, keepdims=True)
    return (
        tw.reshape(_B, _S, _K).astype(np.float32),
        idx.reshape(_B, _S, _K).astype(np.int32),
    )


def _run(x, w_gate, expert_bias, trace=False, mode=None, trace_kwargs=None):
    _ensure_path()
    from concourse.bass_utils import run_bass_kernel_spmd

    nc = _get_program()
    in_maps = _pack_inputs(x, w_gate)
    res = run_bass_kernel_spmd(
        nc, in_maps, list(range(_NCORES)), trace=trace,
        **(trace_kwargs or {}),
    )
    weights, indices = _unpack_outputs(res.results)
    return (weights, indices), res


def kernel(x, w_gate, expert_bias):
    x = np.asarray(x)
    w_gate = np.asarray(w_gate)
    expert_bias = np.asarray(expert_bias)
    assert x.shape == (_B, _S, _D), x.shape
    assert w_gate.shape == (_E, _D), w_gate.shape
    if np.any(expert_bias):
        # Spec pins expert_bias to zeros; keep a correct host path anyway.
        return _numpy_reference(x, w_gate, expert_bias)
    try:
        (weights, indices), _ = _run(x, w_gate, expert_bias)
    except Exception:
        # Transient NRT device wedges have been observed on a first
        # execution; one retry has always recovered.
        import time
        time.sleep(10)
        (weights, indices), _ = _run(x, w_gate, expert_bias)
    return weights, indices


# revision 13
# speedup vs baseline: 1.1421x; 1.1421x over previous
"""MoE routing gate kernel for Trainium2 (8 NeuronCores, data-parallel).

Problem (hardcoded): x [4, 4096, 2048] f32, w_gate [64, 2048] f32,
expert_bias [64] f32 (zeros per spec).
  gate_logits = x @ w_gate.T          # [B, S, 64]
  gate_weights = sigmoid(gate_logits)
  topk_vals, topk_idx = top_k(gate_logits + bias, k=8)
  topk_weights = gather(gate_weights, topk_idx); normalize
Returns (topk_weights [4,4096,8] f32, topk_indices [4,4096,8] int32).

Strategy: shard the 16384 tokens across 8 cores (2048 each); replicate
w_gate. The matmul runs as an exact fp16 hi/lo two-pass scheme instead
of native fp32 (4 cycles/row on the PE): host splits
  x = xh + 2^-11 * xl2,  w = wh + 2^-11 * wl2   (all fp16, exact to
~fp32 precision since fp16 carries 11 mantissa bits per level), and the
device computes, per 128-dim contraction chunk k with ONE stationary
load of S_k = [wh_k | wl2_k] (fp16, full 128-wide PE):
  psum[:, 0:256]   += S_k.T @ xh_k    (tok 0..255 of the group)
  psum[:, 256:512] += S_k.T @ xl2_k
i.e. one 512-row fp16 matmul per k at 1 cycle/row (4x faster than
fp32). Reconstruction (all fp32):
  logits = (psum_hi_rows | xh) + 2^-11*(psum_lo_rows | xh)
         + 2^-11*(psum_hi_rows | xl2)   [+ 2^-22 * (lo|xl2), dropped]
The hi-half combine (same psum partitions) is one DVE
scalar_tensor_tensor; the cross-partition term rides along through the
PE transpose and folds in with a second scalar_tensor_tensor in
token-major layout. Per 128-token tile the DVE max/max_index ops give
top-8 values+indices; ACT sigmoid (+fused row-sum accum), DVE
reciprocal and scalar-mul normalize. Expert bias is zeros per the
problem spec (a numpy fallback guards the general case).
"""

import numpy as np

_B, _S, _D, _E = 4, 4096, 2048, 64
_K = 8
_NCORES = 8
_TOK = _B * _S              # 16384 tokens
_TC = _TOK // _NCORES       # 2048 tokens per core
_GT = 256                   # tokens per group (one psum bank: 2*256 f32)
_NG = _TC // _GT            # 8 token groups per core
_NKC = _D // 128            # 16 contraction chunks
_SPLIT = 2048.0             # 2^11 hi/lo split scale

_prog_cache = {}


def _ensure_path():
    import sys
    for p in ("/opt/trn_rl_repo",):
        if p not in sys.path:
            sys.path.insert(0, p)


def _build_program():
    """Per-core Bass/Tile program (SPMD: same program, different data)."""
    _ensure_path()
    import concourse.bass as bass
    import concourse.tile as tile
    from concourse import bacc, mybir

    nc = bacc.Bacc("TRN2", target_bir_lowering=False, debug=False,
                   num_devices=_NCORES)

    f32 = mybir.dt.float32
    f16 = mybir.dt.float16
    u32 = mybir.dt.uint32
    MULT = mybir.AluOpType.mult
    ADD = mybir.AluOpType.add
    C = 1.0 / _SPLIT

    # DRAM I/O (per core). x layout: [g, dp, k, pass, tau] so each
    # 256-token group streams as one fully-contiguous-per-partition
    # block holding both fp16 passes.
    xg = nc.dram_tensor("xg", [_NG, 128, _NKC, 2, _GT], f16,
                        kind="ExternalInput")
    wt = nc.dram_tensor("wt", [128, _NKC, 128], f16, kind="ExternalInput")
    ident = nc.dram_tensor("ident", [128, 128], f32, kind="ExternalInput")
    out_w = nc.dram_tensor("out_w", [128, _NG, 2, _K], f32,
                           kind="ExternalOutput")
    out_i = nc.dram_tensor("out_i", [128, _NG, 2, _K], u32,
                           kind="ExternalOutput")

    # k-chunk split per group's DMA: fine-grained first loads so the PE
    # starts early; coarser afterwards for DMA efficiency.
    g0_chunks = (1, 1, 2, 4, 8)
    gN_chunks = (8, 8)

    with tile.TileContext(nc) as tc:
        with (
            tc.tile_pool(name="xpool", bufs=4) as xpool,
            tc.tile_pool(name="wpool", bufs=1) as wpool,
            tc.tile_pool(name="psM", bufs=2, space=bass.MemorySpace.PSUM) as psM,
            tc.tile_pool(name="psT", bufs=2, space=bass.MemorySpace.PSUM) as psT,
            tc.tile_pool(name="psW", bufs=1, space=bass.MemorySpace.PSUM) as psW,
            tc.tile_pool(name="cpool", bufs=2) as cpool,
            tc.tile_pool(name="lpool", bufs=2) as lpool,
            tc.tile_pool(name="opool", bufs=2) as opool,
            tc.tile_pool(name="tpool", bufs=4) as tpool,
        ):
            # All loads ride the sync ring in dependency order. The k=0
            # weight slice goes first so the opening matmul gates only
            # on it plus the first 128 KiB x chunk.
            # Weights+ident ride the scalar ring, x rides the sync ring:
            # the two rings generate descriptors in parallel, so neither
            # stream serially delays the other's first use.
            wt_sb = wpool.tile([128, _NKC, 128], f16)
            nc.scalar.dma_start(wt_sb[:, 0:1], wt[:, 0:1])
            xt0 = xpool.tile([128, _NKC, 2, _GT], f16, tag="xg")
            nc.sync.dma_start(xt0[:, 0:1], xg[0][:, 0:1])
            nc.scalar.dma_start(wt_sb[:, 1:], wt[:, 1:])
            nc.sync.dma_start(xt0[:, 1:4], xg[0][:, 1:4])
            id_sb = wpool.tile([128, 128], f32)
            nc.scalar.dma_start(id_sb[:], ident[:])
            nc.sync.dma_start(xt0[:, 4:], xg[0][:, 4:])

            # PE p-state warmup: ~3us of dummy matmuls on zeroed SBUF
            # during the otherwise-dead preamble window, so the tensor
            # engine reaches its top clock before the first real matmul
            # (cold PE runs ~1.5-2.3x slower for the first ~3us busy).
            wu_m = wpool.tile([128, 512], f16)
            nc.gpsimd.memset(wu_m[:], 0.0)
            wu_s = wpool.tile([128, _E], f16)
            nc.gpsimd.memset(wu_s[:], 0.0)
            ps_wu = psW.tile([_E, 512], f32, tag="wu")
            for _ in range(8):
                nc.tensor.matmul(ps_wu[:], wu_s[:], wu_m[:],
                                 start=True, stop=True)

            # Output staging for the whole core; two DMA pairs total.
            wg = opool.tile([128, _NG, 2, _K], f32)
            ig = opool.tile([128, _NG, 2, _K], u32)

            def postprocess(g, cmb):
                """Transpose + top-8 for group g (PE ops emitted AFTER the
                next group's matmuls to avoid head-of-line PE stalls)."""
                pt = psT.tile([128, 2, 128], f32, tag="pt")
                asb = lpool.tile([128, 2, _E], f32, tag="asb")
                lg = lpool.tile([128, 2, _E], f32, tag="lg")
                for j in range(2):
                    nc.tensor.transpose(
                        pt[:, j], cmb[:, bass.ts(j, 128)], id_sb[:],
                    )
                    nc.scalar.copy(asb[:, j], pt[:, j, 0:_E])
                    nc.vector.scalar_tensor_tensor(
                        lg[:, j], pt[:, j, _E:], C, asb[:, j],
                        MULT, ADD,
                    )
                    vals = tpool.tile([128, _K], f32, tag="vals")
                    nc.vector.max(vals[:], lg[:, j])
                    nc.vector.max_index(ig[:, g, j], vals[:], lg[:, j])

                    sig = tpool.tile([128, _K], f32, tag="sig")
                    nc.scalar.activation(
                        sig[:], vals[:], mybir.ActivationFunctionType.Sigmoid,
                    )
                    ssum = tpool.tile([128, 1], f32, tag="ssum")
                    nc.vector.reduce_sum(
                        ssum[:], sig[:], axis=mybir.AxisListType.X,
                    )
                    rsum = tpool.tile([128, 1], f32, tag="rsum")
                    nc.vector.reciprocal(rsum[:], ssum[:])
                    nc.vector.tensor_scalar_mul(wg[:, g, j], sig[:], rsum[:])

            pending = None
            for g in range(_NG):
                xt = xt0 if g == 0 else xpool.tile([128, _NKC, 2, _GT], f16,
                                                   tag="xg")
                if g > 0:
                    # Alternate groups across the two hardware DGE rings.
                    ring = nc.sync if g % 2 == 0 else nc.scalar
                    k0 = 0
                    for nk in gN_chunks:
                        ring.dma_start(xt[:, k0:k0 + nk],
                                       xg[g][:, k0:k0 + nk])
                        k0 += nk
                if g == _NG - 1:
                    # Pre-issue the bulk output DMAs (behind the last x
                    # chunks on the ring) so only the last pair's
                    # descriptor-gen lands in the tail.
                    nc.scalar.dma_start(out_w[:, 0:_NG - 2], wg[:, 0:_NG - 2])
                    nc.scalar.dma_start(out_i[:, 0:_NG - 2], ig[:, 0:_NG - 2])

                # One 512-row fp16 matmul per k: [xh tok | xl2 tok].
                ps = psM.tile([_E * 2, 2 * _GT], f32, tag="ps")
                for k in range(_NKC):
                    nc.tensor.matmul(
                        ps[:], wt_sb[:, k], xt[:, k],
                        start=(k == 0), stop=(k == _NKC - 1),
                    )

                if pending is not None:
                    postprocess(*pending)

                # cmb rows 0:64  = wh.x  (hi|xh + 2^-11 * hi|xl2)
                # cmb rows 64:128 = wl2.xh (folds in post-transpose)
                # (walrus: at most one PSUM input per DVE op, so stage
                # A0 through SBUF on the scalar engine first)
                cmb = cpool.tile([128, _GT], f32, tag="cmb")
                a0 = cpool.tile([_E, _GT], f32, tag="a0")
                nc.scalar.copy(a0[:], ps[0:_E, 0:_GT])
                nc.vector.scalar_tensor_tensor(
                    cmb[0:_E, :], ps[0:_E, _GT:], C, a0[:],
                    MULT, ADD,
                )
                nc.scalar.copy(cmb[_E:, :], ps[_E:, 0:_GT])
                pending = (g, cmb)

            postprocess(*pending)
            nc.scalar.dma_start(out_w[:, _NG - 2:], wg[:, _NG - 2:])
            nc.scalar.dma_start(out_i[:, _NG - 2:], ig[:, _NG - 2:])

    nc.compile()
    return nc


def _get_program():
    if "p" not in _prog_cache:
        _prog_cache["p"] = _build_program()
    return _prog_cache["p"]


def _split_f16(a):
    """a (f32) -> (hi, lo2) fp16 with a ~= hi + lo2/2048 (near-exact)."""
    hi = a.astype(np.float16)
    lo2 = ((a - hi.astype(np.float32)) * _SPLIT).astype(np.float16)
    return hi, lo2


def _pack_inputs(x, w_gate):
    """Host-side layout transform. Returns per-core input maps."""
    x2 = np.ascontiguousarray(x, dtype=np.float32).reshape(_TOK, _D)
    w = np.asarray(w_gate, dtype=np.float32)

    wh, wl2 = _split_f16(w)
    # wt[dp, k, 0:64] = wh[e, k*128+dp]; [.., 64:128] = wl2
    wt = np.empty((128, _NKC, 128), np.float16)
    wt[:, :, 0:_E] = wh.T.reshape(_NKC, 128, _E).transpose(1, 0, 2)
    wt[:, :, _E:] = wl2.T.reshape(_NKC, 128, _E).transpose(1, 0, 2)
    ident = np.eye(128, dtype=np.float32)

    xh, xl2 = _split_f16(x2)

    def pk(a, c):  # core slice -> [g, dp, k, tau]
        ac = a[c * _TC:(c + 1) * _TC]
        return ac.reshape(_NG, _GT, _NKC, 128).transpose(0, 3, 2, 1)

    in_maps = []
    for c in range(_NCORES):
        xgc = np.empty((_NG, 128, _NKC, 2, _GT), np.float16)
        xgc[:, :, :, 0] = pk(xh, c)
        xgc[:, :, :, 1] = pk(xl2, c)
        in_maps.append({"xg": np.ascontiguousarray(xgc), "wt": wt,
                        "ident": ident})
    return in_maps


def _unpack_outputs(results):
    w_parts, i_parts = [], []
    for r in results:
        # [128 tau, 8 g, 2 j, 8] -> token (2g+j)*128+tau -> [2048, 8]
        w_parts.append(
            r["out_w"].reshape(128, _NG * 2, _K).transpose(1, 0, 2).reshape(_TC, _K)
        )
        i_parts.append(
            r["out_i"].reshape(128, _NG * 2, _K).transpose(1, 0, 2).reshape(_TC, _K)
        )
    weights = np.concatenate(w_parts, axis=0).reshape(_B, _S, _K)
    indices = (
        np.concatenate(i_parts, axis=0).astype(np.int32).reshape(_B, _S, _K)
    )
    return weights, indices


def _numpy_reference(x, w_gate, expert_bias):
    """Exact fallback for the (unspecced) nonzero-bias case."""
    x2 = np.asarray(x, dtype=np.float32).reshape(_TOK, _D)
    logits = x2 @ np.asarray(w_gate, dtype=np.float32).T
    gw = 1.0 / (1.0 + np.exp(-logits))
    biased = logits + np.asarray(expert_bias, dtype=np.float32)
    idx = np.argsort(-biased, axis=-1, kind="stable")[:, :_K].astype(np.int32)
    tw = np.take_along_axis(gw, idx, axis=-1)
    tw = tw / tw.sum(axis=-1, keepdims=True)
    return (
        tw.reshape(_B, _S, _K).astype(np.float32),
        idx.reshape(_B, _S, _K).astype(np.int32),
    )


def _run(x, w_gate, expert_bias, trace=False, mode=None, trace_kwargs=None):
    _ensure_path()
    from concourse.bass_utils import run_bass_kernel_spmd

    nc = _get_program()
    in_maps = _pack_inputs(x, w_gate)
    res = run_bass_kernel_spmd(
        nc, in_maps, list(range(_NCORES)), trace=trace,
        **(trace_kwargs or {}),
    )
    weights, indices = _unpack_outputs(res.results)
    return (weights, indices), res


def kernel(x, w_gate, expert_bias):
    x = np.asarray(x)
    w_gate = np.asarray(w_gate)
    expert_bias = np.asarray(expert_bias)
    assert x.shape == (_B, _S, _D), x.shape
    assert w_gate.shape == (_E, _D), w_gate.shape
    if np.any(expert_bias):
        # Spec pins expert_bias to zeros; keep a correct host path anyway.
        return _numpy_reference(x, w_gate, expert_bias)
    try:
        (weights, indices), _ = _run(x, w_gate, expert_bias)
    except Exception:
        # Transient NRT device wedges have been observed on a first
        # execution; one retry has always recovered.
        import time
        time.sleep(10)
        (weights, indices), _ = _run(x, w_gate, expert_bias)
    return weights, indices
